# revision 1
# baseline (speedup 1.0000x reference)
"""Self-contained Trainium2 Bass kernel for the 3-layer GAT problem
(nn_GAT_76166950028493). Accepts FULL inputs, distributes across 8
NeuronCores (dst-range ownership), returns the FULL [40000, 128] output.

Strategy: edges partitioned by dst owner; per-core streams ordered by
128-node dst windows with fixed lo/hi src-index segments (int16 dma_gather
limit); per-layer node tables [ft | el | er] built on device and
all-gathered across cores; segment softmax via denominator-folded
one-hot matmul aggregation in PSUM (max-subtraction-free softmax)."""

import numpy as np
from dataclasses import dataclass

import concourse.bass as bass
import concourse.bacc as bacc
import concourse.tile as tile
from concourse import mybir
from concourse.masks import make_identity
from concourse.bass_utils import run_bass_kernel_spmd

@dataclass
class Cfg:
    N: int = 40000           # real nodes
    NC: int = 8              # cores
    INF: int = 128           # input feats
    HID: int = 64
    HEADS: int = 4
    A1: int = 1024           # lo-seg gather run sizes
    A2: int = 1024
    B: int = 512             # hi-seg run size
    LO: int = 32768          # int16 index limit boundary

    @property
    def PERCORE(self):
        return self.N // self.NC

    @property
    def NPADC(self):          # padded nodes per core
        return ((self.PERCORE + 127) // 128) * 128

    @property
    def NPADG(self):          # padded global nodes
        return self.NPADC * self.NC

    @property
    def WINDOWS(self):
        return self.NPADC // 128

    @property
    def A(self):
        return self.A1 + self.A2

    @property
    def POSW(self):           # positions per window
        return self.A + self.B

    @property
    def TILES_W(self):
        return self.POSW // 128

    @property
    def NPOS(self):
        return self.WINDOWS * self.POSW

    @property
    def NTILES(self):
        return self.NPOS // 128

    @property
    def IDXC(self):
        return self.NPOS // 16


def pad_id(cfg, n):
    """real node id -> padded global id"""
    return cfg.NPADC * (n // cfg.PERCORE) + (n % cfg.PERCORE)


def wrap_runs(cfg, stream_i16):
    """Wrap a per-position int16 index stream into the dma_gather idx layout:
    [128, NPOS/16] where each gather run occupies contiguous cols, idx j of a
    run -> partition j%16 col run_off/16 + j//16, replicated 8x over partition
    groups."""
    out = np.zeros((16, cfg.IDXC), dtype=np.int16)
    runs = []
    for w in range(cfg.WINDOWS):
        base = w * cfg.POSW
        runs += [(base, cfg.A1), (base + cfg.A1, cfg.A2), (base + cfg.A, cfg.B)]
    for off, ln in runs:
        blk = stream_i16[off:off + ln].reshape(ln // 16, 16).T  # [16, ln/16]
        out[:, off // 16:(off + ln) // 16] = blk
    return np.tile(out, (8, 1))  # [128, IDXC]


def build_layout(cfg, src, dst):
    """Per-core edge stream layout. Returns list (per core) of dicts."""
    src = np.asarray(src).astype(np.int64)
    dst = np.asarray(dst).astype(np.int64)
    sp = pad_id(cfg, src)            # padded global src ids
    owner = dst // cfg.PERCORE
    dl = dst % cfg.PERCORE           # dst local id (0..PERCORE)
    cores = []
    for c in range(cfg.NC):
        m = owner == c
        e_sp, e_dl = sp[m], dl[m]
        win = e_dl // 128
        dloc = e_dl % 128
        # per-position streams (init pads: srcidx 0, dstidx 0, dstloc -1)
        srcidx = np.zeros(cfg.NPOS, dtype=np.int64)
        dstidx = np.zeros(cfg.NPOS, dtype=np.int64)
        dstloc = np.full(cfg.NPOS, -1.0, dtype=np.float32)
        eidx = np.full(cfg.NPOS, -1, dtype=np.int64)  # original edge position (emulator)
        for w in range(cfg.WINDOWS):
            wm = win == w
            w_sp, w_dl, w_dloc = e_sp[wm], e_dl[wm], dloc[wm]
            w_ei = np.nonzero(wm)[0]
            lo = w_sp < cfg.LO
            nlo, nhi = int(lo.sum()), int((~lo).sum())
            assert nlo <= cfg.A, f"core {c} win {w}: lo count {nlo} > {cfg.A}"
            assert nhi <= cfg.B, f"core {c} win {w}: hi count {nhi} > {cfg.B}"
            base = w * cfg.POSW
            srcidx[base:base + nlo] = w_sp[lo]
            dstidx[base:base + nlo] = w_dl[lo]
            dstloc[base:base + nlo] = w_dloc[lo]
            eidx[base:base + nlo] = w_ei[lo]
            hb = base + cfg.A
            srcidx[hb:hb + nhi] = w_sp[~lo] - cfg.LO
            dstidx[hb:hb + nhi] = w_dl[~lo]
            dstloc[hb:hb + nhi] = w_dloc[~lo]
            eidx[hb:hb + nhi] = w_ei[~lo]
        cores.append(dict(
            srcidx_w=wrap_runs(cfg, srcidx.astype(np.int16)),
            dstidx_w=wrap_runs(cfg, dstidx.astype(np.int16)),
            dstloc=np.ascontiguousarray(
                dstloc.reshape(cfg.NTILES, 128).T),  # [128, NTILES]
            srcidx=srcidx, dstidx=dstidx, dstloc_flat=dstloc, eidx=eidx,
            edge_mask=m,
        ))
    return cores


def pack_weights(cfg, mlp_c, mlp_t, mlp_d, gat):
    H, HID, INF = cfg.HEADS, cfg.HID, cfg.INF
    f32 = lambda a: np.asarray(a, dtype=np.float32)
    mc = [(f32(W), f32(b)) for W, b in mlp_c]
    mt = [(f32(W), f32(b)) for W, b in mlp_t]
    md = [(f32(W), f32(b)) for W, b in mlp_d]
    g = [{k: f32(v) for k, v in p.items()} for p in gat]

    def m_vec(W, a):
        # M[i,h] = sum_d W[h*D+d, i] * a[h,d]
        D = a.shape[1]
        return np.einsum('hdi,hd->ih', W.reshape(H, D, W.shape[1]), a).astype(np.float32)

    wk = {}
    wk['wsrc'] = np.concatenate([mc[0][0][:, :INF].T, mt[0][0][:, :INF].T,
                                 md[0][0][:, :INF].T], axis=1)  # [128, 192]
    wk['wdst'] = np.concatenate([mc[0][0][:, INF:].T, mt[0][0][:, INF:].T,
                                 md[0][0][:, INF:].T], axis=1)
    wk['b0b'] = np.tile(np.concatenate([mc[0][1], mt[0][1], md[0][1]]), (128, 1)).astype(np.float32)
    wk['w1T'] = np.concatenate([mc[1][0].T, mt[1][0].T, md[1][0].T], axis=1)  # [64, 192]
    wk['b1'] = np.stack([mc[1][1], mt[1][1], md[1][1]], axis=1)  # [64, 3]
    wk['w2T'] = np.concatenate([mc[2][0].T, mt[2][0].T, md[2][0].T], axis=1)  # [64, 384]
    wk['b2'] = np.stack([mc[2][1], mt[2][1], md[2][1]], axis=1)  # [128, 3]
    for l in range(3):
        wk[f'aT{l}'] = np.concatenate(
            [g[l]['ac'].T, g[l]['at'].T, g[l]['ad'].T], axis=1)  # [64or128? -> [D,12]
    # table build mats
    W0, W1, W2 = g[0]['W'], g[1]['W'], g[2]['W']
    w0x = np.zeros((INF, 320), np.float32)
    w0x[:, :256] = W0.T
    w0x[:, 256:260] = m_vec(W0, g[0]['al'])
    w0x[:, 260:264] = m_vec(W0, g[0]['ar'])
    wk['w0x'] = w0x
    w1x = np.zeros((256, 320), np.float32)
    w1x[:, :256] = W1.T
    w1x[:, 256:260] = m_vec(W1, g[1]['al'])
    w1x[:, 260:264] = m_vec(W1, g[1]['ar'])
    wk['w1x'] = w1x
    w2x = np.zeros((256, 576), np.float32)
    w2x[:, :512] = W2.T
    w2x[:, 512:516] = m_vec(W2, g[2]['al'])
    w2x[:, 516:520] = m_vec(W2, g[2]['ar'])
    wk['w2x'] = w2x
    wk['iota'] = np.tile(np.arange(128, dtype=np.float32), (128, 1))
    return wk


def transposed_inputs(cfg, x, cat, time, sdist, ddist):
    """[128, NPADG] transposed padded features + per-core own column slices."""
    def tp(a):
        out = np.zeros((cfg.INF, cfg.NPADG), np.float32)
        a = np.asarray(a, dtype=np.float32)
        for c in range(cfg.NC):
            out[:, c * cfg.NPADC:c * cfg.NPADC + cfg.PERCORE] = \
                a[c * cfg.PERCORE:(c + 1) * cfg.PERCORE].T
        return out
    xT, catT, timeT, sdT, ddT = tp(x), tp(cat), tp(time), tp(sdist), tp(ddist)
    own = lambda a, c: np.ascontiguousarray(a[:, c * cfg.NPADC:(c + 1) * cfg.NPADC])
    return xT, catT, timeT, sdT, ddT, own




F32 = mybir.dt.float32
I16 = mybir.dt.int16
AF = mybir.ActivationFunctionType
OP = mybir.AluOpType
GS = 4  # tiles per chain group in phase A


def build(cfg, debug=False):
    nc = bacc.Bacc("TRN2", target_bir_lowering=False, debug=False,
                   num_devices=cfg.NC)
    NPADG, NPADC, W, TW, NPOS = cfg.NPADG, cfg.NPADC, cfg.WINDOWS, cfg.TILES_W, cfg.NPOS
    POSW, A1, A2, B, LO = cfg.POSW, cfg.A1, cfg.A2, cfg.B, cfg.LO
    runs = [(0, A1, 0), (A1, A2, A1 // 128), (A1 + A2, B, (A1 + A2) // 128)]

    di = {}

    def din(name, shape, dt=F32):
        di[name] = nc.dram_tensor(name, shape, dt, kind="ExternalInput")
        return di[name]

    xT = din("xT", [128, NPADG])
    featT = din("featT", [128, 3, NPADG])
    xT_own = din("xT_own", [128, NPADC])
    featT_own = din("featT_own", [128, 3, NPADC])
    srcidx = din("srcidx", [128, cfg.IDXC], I16)
    dstidx = din("dstidx", [128, cfg.IDXC], I16)
    dstloc = din("dstloc", [128, cfg.NTILES])
    wsrc = din("wsrc", [128, 192])
    wdst = din("wdst", [128, 192])
    w1T = din("w1T", [64, 192])
    b1 = din("b1", [64, 3])
    w2T = din("w2T", [64, 384])
    b2 = din("b2", [128, 3])
    aT0 = din("aT0", [64, 12])
    aT1 = din("aT1", [64, 12])
    aT2 = din("aT2", [128, 12])
    w0x = din("w0x", [128, 320])
    w1x = din("w1x", [256, 320])
    w2x = din("w2x", [256, 576])
    iota = din("iota", [128, 128])
    b0f = din("b0f", [64, 3])

    out = nc.dram_tensor("out", [NPADC, 128], F32, kind="ExternalOutput")
    dbg = {}
    if debug:
        dbg['S'] = nc.dram_tensor("S_dbg", [NPOS, 12], F32, kind="ExternalOutput")
        dbg['h0'] = nc.dram_tensor("h0_dbg", [NPADC, 256], F32, kind="ExternalOutput")

    proj_src = nc.dram_tensor("proj_src", [NPADG, 192], F32)
    pdst_sh = nc.dram_tensor("pdst_sh", [NPADC + 1, 192], F32)
    tb0 = nc.dram_tensor("tb0", [NPADG, 320], F32)
    tb0_sh = nc.dram_tensor("tb0_sh", [NPADC + 1, 320], F32)
    tb1_sh = nc.dram_tensor("tb1_sh", [NPADC + 1, 320], F32)
    tb2_sh = nc.dram_tensor("tb2_sh", [NPADC + 1, 576], F32)
    aspace = "Shared" if cfg.NC > 4 else "Local"
    tb1 = nc.dram_tensor("tb1", [NPADG, 320], F32, addr_space=aspace)
    tb2 = nc.dram_tensor("tb2", [NPADG, 576], F32, addr_space=aspace)
    S_str = nc.dram_tensor("S_str", [NPOS, 12], F32)

    with tile.TileContext(nc) as tc:
        # ---------------- stage 0: node tables ----------------
        with (
            tc.tile_pool(name="s0w", bufs=1) as s0w,
            tc.tile_pool(name="s0", bufs=3) as s0,
            tc.tile_pool(name="s0p", bufs=3, space="PSUM") as s0p,
        ):
            wsrc_t = s0w.tile([128, 192], F32, tag="wsrc")
            nc.sync.dma_start(out=wsrc_t[:], in_=wsrc[:])
            wdst_t = s0w.tile([128, 192], F32, tag="wdst")
            nc.sync.dma_start(out=wdst_t[:], in_=wdst[:])
            w0x_t = s0w.tile([128, 320], F32, tag="w0x")
            nc.sync.dma_start(out=w0x_t[:], in_=w0x[:])

            def proj_chunks(src_feat, src_x, n_chunks, dst_proj, dst_tb, wproj):
                for ch in range(n_chunks):
                    sl = slice(ch * 128, (ch + 1) * 128)
                    ft = s0.tile([128, 3, 128], F32, tag="ft")
                    nc.sync.dma_start(out=ft[:], in_=src_feat[:, :, sl])
                    xt = s0.tile([128, 128], F32, tag="xt")
                    nc.sync.dma_start(out=xt[:], in_=src_x[:, sl])
                    pp = s0p.tile([128, 192], F32, tag="pp")
                    for k in range(3):
                        nc.tensor.matmul(pp[:, k * 64:(k + 1) * 64], lhsT=ft[:, k, :],
                                         rhs=wproj[:, k * 64:(k + 1) * 64],
                                         start=True, stop=True)
                    ps = s0.tile([128, 192], F32, tag="ps")
                    nc.scalar.activation(ps[:], pp[:], AF.Copy)
                    nc.sync.dma_start(out=dst_proj[sl, :], in_=ps[:])
                    tp = s0p.tile([128, 320], F32, tag="tp")
                    nc.tensor.matmul(tp[:], lhsT=xt[:], rhs=w0x_t[:], start=True, stop=True)
                    ts = s0.tile([128, 320], F32, tag="ts")
                    nc.scalar.activation(ts[:], tp[:], AF.Copy)
                    nc.sync.dma_start(out=dst_tb[sl, :], in_=ts[:])

            proj_chunks(featT, xT, NPADG // 128, proj_src, tb0, wsrc_t)
            proj_chunks(featT_own, xT_own, NPADC // 128, pdst_sh, tb0_sh, wdst_t)

        with tc.tile_pool(name="res", bufs=1) as res:
            si_t = res.tile([128, cfg.IDXC], I16, tag="si")
            nc.sync.dma_start(out=si_t[:], in_=srcidx[:])
            di_t = res.tile([128, cfg.IDXC], I16, tag="di")
            nc.sync.dma_start(out=di_t[:], in_=dstidx[:])
            dl_t = res.tile([128, cfg.NTILES], F32, tag="dl")
            nc.sync.dma_start(out=dl_t[:], in_=dstloc[:])
            iota_t = res.tile([128, 128], F32, tag="iota")
            nc.sync.dma_start(out=iota_t[:], in_=iota[:])
            w1T_t = res.tile([64, 192], F32, tag="w1T")
            nc.sync.dma_start(out=w1T_t[:], in_=w1T[:])
            b1_t = res.tile([64, 3], F32, tag="b1")
            nc.sync.dma_start(out=b1_t[:], in_=b1[:])
            w2T_t = res.tile([64, 384], F32, tag="w2T")
            nc.sync.dma_start(out=w2T_t[:], in_=w2T[:])
            b2_t = res.tile([128, 3], F32, tag="b2")
            nc.sync.dma_start(out=b2_t[:], in_=b2[:])
            aT0_t = res.tile([64, 12], F32, tag="aT0")
            aT1_t = res.tile([64, 12], F32, tag="aT1")
            aT2_t = res.tile([128, 12], F32, tag="aT2")
            aT_t = [aT0_t, aT1_t, aT2_t]
            for l, t in enumerate(aT_t):
                nc.sync.dma_start(out=t[:], in_=di[f"aT{l}"][:])
            b0f_t = res.tile([64, 3], F32, tag="b0f")
            nc.sync.dma_start(out=b0f_t[:], in_=b0f[:])
            w1x_a = res.tile([128, 320], F32, tag="w1xa")
            w1x_b = res.tile([128, 320], F32, tag="w1xb")
            nc.sync.dma_start(out=w1x_a[:], in_=w1x[0:128, :])
            nc.sync.dma_start(out=w1x_b[:], in_=w1x[128:256, :])
            w2x_a = res.tile([128, 576], F32, tag="w2xa")
            w2x_b = res.tile([128, 576], F32, tag="w2xb")
            nc.sync.dma_start(out=w2x_a[:], in_=w2x[0:128, :])
            nc.sync.dma_start(out=w2x_b[:], in_=w2x[128:256, :])
            ident = res.tile([128, 128], F32, tag="ident")
            make_identity(nc, ident[:])

            def gather_win(w, elem, step, dst_tile, idx_tile, views):
                """views: (lo_view, hi_view) or single view for all runs."""
                for (poff, ln, t0) in runs:
                    goff = (w * POSW + poff) // 16
                    if isinstance(views, tuple):
                        view = views[0] if poff < A1 + A2 else views[1]
                    else:
                        view = views
                    nc.gpsimd.dma_gather(
                        out_ap=dst_tile[:, t0:t0 + ln // 128, :],
                        in_ap=view, idxs_ap=idx_tile[:, goff:goff + ln // 16],
                        num_idxs=ln, num_idxs_reg=ln,
                        elem_size=elem, elem_step=step)

            # ---------------- phase A ----------------
            with (
                tc.tile_pool(name="ga", bufs=2) as ga,
                tc.tile_pool(name="wa", bufs=2) as wa,
                tc.tile_pool(name="er", bufs=1) as erp,
                tc.tile_pool(name="ecp", bufs=2) as ecp,
                tc.tile_pool(name="pqt", bufs=2, space="PSUM") as pqt,
                tc.tile_pool(name="psp", bufs=2, space="PSUM") as psp,
                tc.tile_pool(name="pm", bufs=2, space="PSUM") as pm,
                tc.tile_pool(name="pst", bufs=2, space="PSUM") as pst,
            ):
                for w in range(W):
                    pa_t = ga.tile([128, TW, 192], F32, tag="paw")
                    gather_win(w, 192, 192, pa_t, si_t,
                               (proj_src[0:LO, :], proj_src[LO:NPADG, :]))
                    pb_t = ga.tile([128, TW, 192], F32, tag="pbw")
                    gather_win(w, 192, 192, pb_t, di_t, pdst_sh[0:NPADC, :])
                    nc.vector.tensor_add(pa_t[:], pa_t[:], pb_t[:])  # q in-place
                    S_win = wa.tile([128, TW, 12], F32, tag="swin")
                    for g in range(TW // GS):
                        NG = 128 * GS
                        ecT = [None, None, None]
                        for chn in range(3):
                            qT = pqt.tile([64, NG], F32, tag="qT")
                            for t in range(GS):
                                nc.tensor.transpose(
                                    qT[:, t * 128:(t + 1) * 128],
                                    pa_t[:, g * GS + t, chn * 64:(chn + 1) * 64],
                                    ident[:])
                            e_ = erp.tile([64, NG], F32, tag="e0")
                            nc.scalar.activation(e_[:], qT[:], AF.Exp,
                                                 bias=b0f_t[:, chn:chn + 1])
                            r_ = erp.tile([64, NG], F32, tag="r0")
                            nc.scalar.activation(r_[:], qT[:], AF.Relu,
                                                 bias=b0f_t[:, chn:chn + 1])
                            nc.vector.tensor_scalar(e_[:], e_[:], 1.0, -1.0,
                                                    OP.min, OP.add)
                            ec = ecp.tile([64, NG], F32, tag=f"ec0_{chn}")
                            nc.vector.tensor_add(ec[:], e_[:], r_[:])
                            ecT[chn] = ec
                        sls = []
                        for l in range(3):
                            sp = psp.tile([4, NG], F32, tag="sp")
                            for chn in range(3):
                                nc.tensor.matmul(sp[:], lhsT=aT_t[l][:, chn * 4:(chn + 1) * 4],
                                                 rhs=ecT[chn][:], start=(chn == 0), stop=(chn == 2))
                            sl_t = wa.tile([4, NG], F32, tag=f"sl{l}")
                            nc.vector.tensor_copy(sl_t[:], sp[:])
                            sls.append(sl_t)
                            if l == 2:
                                break
                            nxt = [None, None, None]
                            for chn in range(3):
                                if l == 0:
                                    mp = pm.tile([64, NG], F32, tag="m")
                                    nc.tensor.matmul(mp[:], lhsT=w1T_t[:, chn * 64:(chn + 1) * 64],
                                                     rhs=ecT[chn][:], start=True, stop=True)
                                    bias, shape = b1_t[:, chn:chn + 1], [64, NG]
                                else:
                                    mp = pm.tile([128, NG], F32, tag="m")
                                    nc.tensor.matmul(mp[:], lhsT=w2T_t[:, chn * 128:(chn + 1) * 128],
                                                     rhs=ecT[chn][:], start=True, stop=True)
                                    bias, shape = b2_t[:, chn:chn + 1], [128, NG]
                                e_ = erp.tile(shape, F32, tag="e1")
                                nc.scalar.activation(e_[:], mp[:], AF.Exp, bias=bias)
                                r_ = erp.tile(shape, F32, tag="r1")
                                nc.scalar.activation(r_[:], mp[:], AF.Relu, bias=bias)
                                nc.vector.tensor_scalar(e_[:], e_[:], 1.0, -1.0,
                                                        OP.min, OP.add)
                                ec = ecp.tile(shape, F32, tag=f"ec{l + 1}_{chn}")
                                nc.vector.tensor_add(ec[:], e_[:], r_[:])
                                nxt[chn] = ec
                            ecT = nxt
                        for t in range(GS):
                            for l in range(3):
                                stp = pst.tile([128, 4], F32, tag="stp")
                                nc.tensor.transpose(stp[:], sls[l][:, t * 128:(t + 1) * 128],
                                                    ident[0:4, 0:4])
                                nc.vector.tensor_copy(S_win[:, g * GS + t, 4 * l:4 * l + 4], stp[:])
                    sview = S_str[w * POSW:(w + 1) * POSW, :].rearrange(
                        "(t p) s -> p t s", p=128)
                    nc.sync.dma_start(out=sview, in_=S_win[:])
                    if debug:
                        dview = dbg['S'][w * POSW:(w + 1) * POSW, :].rearrange(
                            "(t p) s -> p t s", p=128)
                        nc.sync.dma_start(out=dview, in_=S_win[:])

            # ---------------- GAT layers ----------------
            with (
                tc.tile_pool(name="gl", bufs=2) as gl,
                tc.tile_pool(name="wl", bufs=2) as wl,
                tc.tile_pool(name="pag", bufs=2, space="PSUM") as pag,
                tc.tile_pool(name="pad", bufs=2, space="PSUM") as pad,
                tc.tile_pool(name="pht", bufs=1, space="PSUM") as pht,
                tc.tile_pool(name="pbt", bufs=1, space="PSUM") as pbt,
            ):
                def layer(l, tbl_full, tbl_sh, width, wxa, wxb, next_sh):
                    ftw = 256 if l < 2 else 512
                    hd = ftw // 4
                    for w in range(W):
                        ft_t = gl.tile([128, TW, width], F32, tag="ftw")
                        gather_win(w, width, width, ft_t, si_t,
                                   (tbl_full[0:LO, :], tbl_full[LO:NPADG, :]))
                        aux_t = gl.tile([128, TW, 64], F32, tag="aux")
                        gather_win(w, 64, width, aux_t, di_t,
                                   tbl_sh[0:NPADC, ftw:ftw + 64])
                        S_win = wl.tile([128, TW, 12], F32, tag="swin2")
                        sview = S_str[w * POSW:(w + 1) * POSW, :].rearrange(
                            "(t p) s -> p t s", p=128)
                        nc.sync.dma_start(out=S_win[:], in_=sview)
                        z_t = wl.tile([128, TW, 4], F32, tag="zw")
                        nc.vector.tensor_add(z_t[:], S_win[:, :, 4 * l:4 * l + 4],
                                             ft_t[:, :, ftw:ftw + 4])
                        nc.vector.tensor_add(z_t[:], z_t[:], aux_t[:, :, 4:8])
                        zs_t = wl.tile([128, TW, 4], F32, tag="zs")
                        nc.vector.tensor_scalar(zs_t[:], z_t[:], 0.2, None, OP.mult)
                        nc.vector.tensor_tensor(out=z_t[:], in0=z_t[:], in1=zs_t[:], op=OP.max)
                        nc.scalar.activation(z_t[:], z_t[:], AF.Exp)
                        for h in range(4):
                            nc.vector.tensor_tensor(
                                out=ft_t[:, :, h * hd:(h + 1) * hd],
                                in0=ft_t[:, :, h * hd:(h + 1) * hd],
                                in1=z_t[:, :, h:h + 1].to_broadcast([128, TW, hd]),
                                op=OP.mult)
                        agg = pag.tile([128, ftw], F32, tag="agg")
                        aggd = pad.tile([128, 4], F32, tag="aggd")
                        for t in range(TW):
                            sel = wl.tile([128, 128], F32, tag="sel")
                            nc.vector.tensor_tensor(
                                out=sel[:],
                                in0=dl_t[:, w * TW + t:w * TW + t + 1].to_broadcast([128, 128]),
                                in1=iota_t[:], op=OP.is_equal)
                            nc.tensor.matmul(agg[:], lhsT=sel[:], rhs=ft_t[:, t, 0:ftw],
                                             start=(t == 0), stop=(t == TW - 1))
                            nc.tensor.matmul(aggd[:], lhsT=sel[:], rhs=z_t[:, t, :],
                                             start=(t == 0), stop=(t == TW - 1))
                        inv = wl.tile([128, 4], F32, tag="inv")
                        if l < 2:
                            nc.vector.tensor_scalar(inv[:], aggd[:], 1e-30, None, OP.max)
                        else:
                            nc.vector.tensor_scalar(inv[:], aggd[:], 1e-30, 4.0,
                                                    OP.max, OP.mult)
                        nc.vector.reciprocal(inv[:], inv[:])
                        h_t = wl.tile([128, ftw], F32, tag="hw")
                        for h in range(4):
                            nc.vector.tensor_tensor(
                                out=h_t[:, h * hd:(h + 1) * hd],
                                in0=agg[:, h * hd:(h + 1) * hd],
                                in1=inv[:, h:h + 1].to_broadcast([128, hd]),
                                op=OP.mult)
                        if l < 2:
                            for _ in range(2):
                                e_ = wl.tile([128, ftw], F32, tag="he")
                                nc.scalar.activation(e_[:], h_t[:], AF.Exp)
                                r_ = wl.tile([128, ftw], F32, tag="hr")
                                nc.scalar.activation(r_[:], h_t[:], AF.Relu)
                                nc.vector.tensor_scalar(e_[:], e_[:], 1.0, -1.0,
                                                        OP.min, OP.add)
                                nc.vector.tensor_add(h_t[:], e_[:], r_[:])
                            if debug and l == 0:
                                nc.sync.dma_start(out=dbg['h0'][w * 128:(w + 1) * 128, :],
                                                  in_=h_t[:])
                            hTs = []
                            for k in range(2):
                                hT = pht.tile([128, 128], F32, tag="hT")
                                nc.tensor.transpose(hT[:], h_t[:, k * 128:(k + 1) * 128], ident[:])
                                hs = wl.tile([128, 128], F32, tag=f"hTs{k}")
                                nc.vector.tensor_copy(hs[:], hT[:])
                                hTs.append(hs)
                            nw = 320 if l == 0 else 576
                            n1 = min(nw, 512)
                            bt = pbt.tile([128, n1], F32, tag="bt")
                            nc.tensor.matmul(bt[:], lhsT=hTs[0][:], rhs=wxa[:, 0:n1],
                                             start=True, stop=False)
                            nc.tensor.matmul(bt[:], lhsT=hTs[1][:], rhs=wxb[:, 0:n1],
                                             start=False, stop=True)
                            bts = wl.tile([128, nw], F32, tag="bts")
                            nc.scalar.activation(bts[:, 0:n1], bt[:], AF.Copy)
                            if nw > 512:
                                bt2 = pbt.tile([128, nw - 512], F32, tag="bt2")
                                nc.tensor.matmul(bt2[:], lhsT=hTs[0][:], rhs=wxa[:, 512:nw],
                                                 start=True, stop=False)
                                nc.tensor.matmul(bt2[:], lhsT=hTs[1][:], rhs=wxb[:, 512:nw],
                                                 start=False, stop=True)
                                nc.scalar.activation(bts[:, 512:nw], bt2[:], AF.Copy)
                            nc.sync.dma_start(out=next_sh[w * 128:(w + 1) * 128, :], in_=bts[:])
                        else:
                            o_t = wl.tile([128, 128], F32, tag="ow")
                            nc.vector.tensor_tensor(out=o_t[:], in0=h_t[:, 0:128],
                                                    in1=h_t[:, 128:256], op=OP.add)
                            nc.vector.tensor_tensor(out=o_t[:], in0=o_t[:],
                                                    in1=h_t[:, 256:384], op=OP.add)
                            nc.vector.tensor_tensor(out=o_t[:], in0=o_t[:],
                                                    in1=h_t[:, 384:512], op=OP.add)
                            nc.sync.dma_start(out=out[w * 128:(w + 1) * 128, :], in_=o_t[:])

                layer(0, tb0, tb0_sh, 320, w1x_a, w1x_b, tb1_sh)
                nc.gpsimd.collective_compute(
                    "AllGather", OP.bypass,
                    replica_groups=[list(range(cfg.NC))],
                    ins=[tb1_sh[0:NPADC, :].opt()], outs=[tb1[:, :].opt()])
                layer(1, tb1, tb1_sh, 320, w2x_a, w2x_b, tb2_sh)
                nc.gpsimd.collective_compute(
                    "AllGather", OP.bypass,
                    replica_groups=[list(range(cfg.NC))],
                    ins=[tb2_sh[0:NPADC, :].opt()], outs=[tb2[:, :].opt()])
                layer(2, tb2, tb2_sh, 576, None, None, None)

    nc.compile()
    return nc


def make_in_maps(cfg, layout, wk, inputs):
    x = np.asarray(inputs['x'], np.float32)
    cat = np.asarray(inputs['cat_feat'], np.float32)
    tim = np.asarray(inputs['time_feat'], np.float32)
    sd = np.asarray(inputs['src_dist_feat'], np.float32)
    dd = np.asarray(inputs['dst_dist_feat'], np.float32)
    xT, catT, timeT, sdT, ddT, own = transposed_inputs(cfg, x, cat, tim, sd, dd)
    featT = np.ascontiguousarray(np.stack([catT, timeT, sdT], axis=1))
    featT_own_g = np.stack([catT, timeT, ddT], axis=1)
    b0f = np.stack([np.asarray(inputs['mlp_c'][0][1], np.float32),
                    np.asarray(inputs['mlp_t'][0][1], np.float32),
                    np.asarray(inputs['mlp_d'][0][1], np.float32)], axis=1)
    common = dict(
        xT=xT, featT=featT,
        wsrc=wk['wsrc'], wdst=wk['wdst'], w1T=wk['w1T'], b1=wk['b1'],
        w2T=wk['w2T'], b2=wk['b2'], aT0=wk['aT0'], aT1=wk['aT1'], aT2=wk['aT2'],
        w0x=wk['w0x'], w1x=wk['w1x'], w2x=wk['w2x'], iota=wk['iota'], b0f=b0f,
    )
    maps = []
    for c in range(cfg.NC):
        m = dict(common)
        m['xT_own'] = np.ascontiguousarray(xT[:, c * cfg.NPADC:(c + 1) * cfg.NPADC])
        m['featT_own'] = np.ascontiguousarray(
            featT_own_g[:, :, c * cfg.NPADC:(c + 1) * cfg.NPADC])
        m['srcidx'] = layout[c]['srcidx_w']
        m['dstidx'] = layout[c]['dstidx_w']
        m['dstloc'] = layout[c]['dstloc']
        maps.append(m)
    return maps


_CACHE = {}


def kernel(**inputs) -> np.ndarray:
    cfg = Cfg()
    inp = {}
    for k in ['x', 'cat_feat', 'time_feat', 'src_dist_feat', 'dst_dist_feat',
              'src', 'dst']:
        inp[k] = np.asarray(inputs[k])
    full = dict(inp)
    for k in ['mlp_c', 'mlp_t', 'mlp_d']:
        full[k] = [(np.asarray(W), np.asarray(b)) for W, b in inputs[k]]
    full['gat'] = [{kk: np.asarray(vv) for kk, vv in p.items()} for p in inputs['gat']]

    layout = build_layout(cfg, inp['src'], inp['dst'])
    wk = pack_weights(cfg, full['mlp_c'], full['mlp_t'], full['mlp_d'], full['gat'])
    if 'nc' not in _CACHE:
        _CACHE['nc'] = build(cfg, debug=False)
    nc = _CACHE['nc']
    maps = make_in_maps(cfg, layout, wk, full)
    res = run_bass_kernel_spmd(nc, maps, list(range(cfg.NC)))
    out = np.concatenate(
        [res.results[c]["out"][:cfg.PERCORE] for c in range(cfg.NC)], axis=0)
    return out.astype(np.float32)
